# revision 13
# baseline (speedup 1.0000x reference)
"""Causal self-attention Trainium2 Bass kernel.

Shapes (hardcoded): B=8, T=1024, C=768, NH=12, HS=64.
Sharding: data-parallel over batch — core b computes batch element b.

Per-core dataflow (all matmuls bf16 with fp32 PSUM accumulation):
  - DMA order on the sync ring: wqk pair0, tri mask, xT in 6 k-slabs,
    wv, remaining wqk pairs, wp — so the first QK projection matmul can
    start as soon as slab k0 lands. A dozen scratch warmup matmuls run
    during the DMA wait to bring the PE HAM clock-gate to 8/8.
  - qkT  [2C, T] channel-major  = w_qk.T-tiles (stationary) x xT (moving).
    Pair hp+1's QK projection is emitted between chunk 0 and chunk 1 of
    pair hp so its ScalarE copies are never behind a full exp queue.
  - v    token-major [T, C], assembled into v_aug [jt, head, 65] with a
    ones column so the PV matmul also emits softmax row-sums for free
  - S^T  [j, i] blocks per head: lhsT = kT j-tile (K=64), rhs = qT i-cols.
    The two heads of a pair occupy PE row-groups 0-1 / 2-3 via explicit
    tile_position (0,0)/(64,0): issued back-to-back they stream
    concurrently (measured 2.06x).
  - Causality: block skipping; diagonal-block masking is a bf16 DVE
    multiply of P^T by a lower-triangular 0/1 mask after the exp (keeps
    128-row matmuls out of the 64-row tiled S runs — mode switches
    drain the PE array).
  - exp via ScalarE activation (scale=1/8) PSUM->SBUF into bf16 P^T.
    Emission is software-pipelined: PV of group g-1 is interleaved
    between S^T groups so the in-order PE queue never waits on exp.
  - y^T [65, i] = v_aug.T x P^T accumulated over j-tiles in PSUM; row 64
    is the softmax denominator. One DVE copy frees the PSUM slot; then
    sums-row copy, DVE reciprocal_approx_fast, gpsimd
    partition_broadcast, DVE multiply into yT [C, T] bf16.
  - out [T, C] = yT-tiles (stationary) x w_proj (moving), DVE copy, DMA.
"""

import numpy as np

import concourse.bass as bass
import concourse.mybir as mybir
import concourse.tile as tile
from concourse import bacc
from concourse.bass_utils import run_bass_kernel_spmd

B, T, C = 8, 1024, 768
NH, HS = 12, 64
NCORES = 8
KT = C // 128            # 6 contraction tiles
NPAIR = NH // 2          # 6 head pairs; head-pair hp covers heads 2hp, 2hp+1
F32 = mybir.dt.float32
BF16 = mybir.dt.bfloat16

_cache = {}


def _build_program(bias_attn: bool, bias_proj: bool):
    nc = bacc.Bacc("TRN2", target_bir_lowering=False, debug=False,
                   num_devices=NCORES)

    xT = nc.dram_tensor("xT", [C, T], BF16, kind="ExternalInput")
    wqk = nc.dram_tensor("wqk", [C, 2 * C], BF16, kind="ExternalInput")
    wv = nc.dram_tensor("wv", [C, 780], BF16, kind="ExternalInput")
    wp = nc.dram_tensor("wp", [C, C], BF16, kind="ExternalInput")
    if bias_attn:
        bqk_d = nc.dram_tensor("bqk", [2 * C], F32, kind="ExternalInput")
        bv_d = nc.dram_tensor("bv", [780], F32, kind="ExternalInput")
    if bias_proj:
        bp_d = nc.dram_tensor("bp", [C], F32, kind="ExternalInput")
    out = nc.dram_tensor("out", [T, C], F32, kind="ExternalOutput")

    # 0/1 causal (j <= i) mask for P^T diagonal blocks, duplicated for
    # the two heads sharing a P^T tile ([128, hl, 128]).
    import ml_dtypes
    tri_np = np.broadcast_to(
        np.tril(np.ones((128, 128))).T[:, None, :], (128, 2, 128)
    ).astype(ml_dtypes.bfloat16)
    tri_d = nc.inline_tensor(np.ascontiguousarray(tri_np), "tri01")

    xT_r = xT.ap().rearrange("(k p) t -> p k t", p=128)
    wqk_r = wqk.ap().rearrange("(k p) m -> p k m", p=128)
    wv_r = wv.ap().rearrange("(k p) m -> p k m", p=128)
    wp_r = wp.ap().rearrange("(k p) m -> p k m", p=128)

    with tile.TileContext(nc) as tc:
        with (
            tc.tile_pool(name="xpool", bufs=1) as xpool,
            tc.tile_pool(name="cpool", bufs=1) as cpool,
            tc.tile_pool(name="wvpool", bufs=1) as wvpool,
            tc.tile_pool(name="vpool", bufs=1) as vpool,
            tc.tile_pool(name="wqkpool", bufs=6) as wqkpool,
            tc.tile_pool(name="qkpool", bufs=6) as qkpool,
            tc.tile_pool(name="ptpool", bufs=8) as ptpool,
            tc.tile_pool(name="ytpool", bufs=1) as ytpool,
            tc.tile_pool(name="wppool", bufs=2) as wppool,
            tc.tile_pool(name="opool", bufs=3) as opool,
            tc.tile_pool(name="smpool", bufs=6) as smpool,
            # psA: dedicated S^T ring (2 x 2 banks) — the dense
            # projections never sit in it, so the attention pipeline
            # depends only on the exp drain rate. psC: 2 x 1-bank slots
            # for the QKV/V/proj matmuls at 512-column granularity
            # (double-buffered: the PSUM->SBUF copy of one group
            # overlaps the matmuls of the next). psB: y accumulators.
            tc.tile_pool(name="psA", bufs=2, space="PSUM") as psA,
            tc.tile_pool(name="psC", bufs=2, space="PSUM") as psC,
            tc.tile_pool(name="psB", bufs=1, space="PSUM") as psB,
        ):
            # ---- tile allocations whose DMAs are sequenced below ----
            xT_s = xpool.tile([128, KT, T], BF16, tag="xT")
            wv_s = wvpool.tile([128, KT, 780], BF16, tag="wv")

            # ---- PE warmup: garbage matmuls during the input DMA wait
            # keep the HAM activity window busy so real matmuls run at
            # 2.4 GHz from the start.
            warm = cpool.tile([128, 640], BF16, tag="warm")
            nc.vector.memset(warm[:], 0.125)
            wps0 = psC.tile([128, 512], F32, tag="pj", name="warmps")
            for r in range(12):
                nc.tensor.matmul(wps0[:], warm[:, 512:640],
                                 warm[:, 0:512], start=True, stop=True)

            def emit_qkdma(hp):
                wt = wqkpool.tile([128, KT, 256], BF16, tag="wqk",
                                  name=f"wt_{hp}")
                nc.sync.dma_start(wt[:, :, 0:128],
                                  wqk_r[:, :, hp * 128:(hp + 1) * 128])
                nc.sync.dma_start(wt[:, :, 128:256],
                                  wqk_r[:, :, C + hp * 128:C + (hp + 1) * 128])
                return wt

            # ---- DMA issue order (sync ring is FIFO): wqk pair0 first,
            # then the small tri mask, then xT k-slabs (the QK projection
            # consumes them k-inner, so matmuls start after slab 0), wv,
            # the remaining wqk pairs, wp.
            wts = [emit_qkdma(0)]
            tri2_s = cpool.tile([128, 2, 128], BF16, tag="tri01")
            nc.sync.dma_start(tri2_s[:], tri_d.ap())
            if bias_attn:
                bqk_s = cpool.tile([128, 12], F32, tag="bqk")
                nc.sync.dma_start(bqk_s[:], bqk_d.ap().rearrange("(m p) -> p m", p=128))
            for k in range(KT):
                nc.sync.dma_start(xT_s[:, k, :], xT_r[:, k, :])
            nc.sync.dma_start(wv_s[:, :, 0:390], wv_r[:, :, 0:390])
            nc.sync.dma_start(wv_s[:, :, 390:780], wv_r[:, :, 390:780])
            if bias_attn:
                bv_row = cpool.tile([1, 780], F32, tag="bvrow")
                nc.sync.dma_start(bv_row[:], bv_d.ap().rearrange("c -> 1 c"))
                bv_bc = cpool.tile([128, 780], F32, tag="bvbc")
                nc.gpsimd.partition_broadcast(bv_bc[:], bv_row[:])
            if bias_proj:
                bp_row = cpool.tile([1, C], F32, tag="bprow")
                nc.sync.dma_start(bp_row[:], bp_d.ap().rearrange("c -> 1 c"))
                bp_bc = cpool.tile([128, C], F32, tag="bpbc")
                nc.gpsimd.partition_broadcast(bp_bc[:], bp_row[:])
            for hp in range(1, NPAIR):
                wts.append(emit_qkdma(hp))

            def emit_qkmm(hp):
                wt = wts[hp]
                qk_t = qkpool.tile([128, 2, T], BF16, tag="qk",
                                   name=f"qk_{hp}")
                for part in range(2):  # 0 = q m-tile hp, 1 = k m-tile hp
                    for nch in range(2):
                        ps = psC.tile([128, 512], F32, tag="pj",
                                      name=f"qkps_{hp}_{part}_{nch}")
                        for k in range(KT):
                            nc.tensor.matmul(
                                ps[:],
                                wt[:, k, part * 128:part * 128 + 128],
                                xT_s[:, k, nch * 512:(nch + 1) * 512],
                                start=(k == 0), stop=(k == KT - 1),
                            )
                        # PSUM->SBUF on ScalarE, one 512-half at a time:
                        # exps can slot between the halves and the PSUM
                        # slot frees sooner.
                        half = qk_t[:, part, nch * 512:(nch + 1) * 512]
                        if bias_attn:
                            nc.scalar.add(half, ps[:],
                                          bqk_s[:, part * 6 + hp:part * 6 + hp + 1])
                        else:
                            nc.scalar.copy(half, ps[:])
                return qk_t

            qk_next = emit_qkmm(0)

            # ---- V: token-major, assembled as v_aug[jt, pair, hl, 65] ----
            # Each head block is [v(64) | ones(1)]; wv is zero-padded
            # host-side to 65-col head blocks so one copy per (jt, group)
            # lands v in place, then the ones cols are re-memset (the
            # copy writes wv-pad zeros there).
            v_aug = vpool.tile([128, 8, NPAIR, 2, HS + 1], BF16, tag="vaug")
            for jt in range(8):
                for p0, off in ((0, 0), (3, 390)):
                    ps = psC.tile([128, 512], F32, tag="pj",
                                  name=f"vps_{jt}_{off}")
                    for k in range(KT):
                        nc.tensor.matmul(
                            ps[:, 0:390],
                            xT_s[:, k, jt * 128:(jt + 1) * 128],
                            wv_s[:, k, off:off + 390],
                            start=(k == 0), stop=(k == KT - 1),
                        )
                    dst = v_aug[:, jt, p0:p0 + 3, :, :]
                    src = ps[:, 0:390].rearrange("p (r h c) -> p r h c",
                                                 r=3, h=2)
                    if bias_attn:
                        nc.vector.tensor_add(
                            dst, src,
                            bv_bc[:, off:off + 390].rearrange(
                                "p (r h c) -> p r h c", r=3, h=2))
                    else:
                        nc.vector.tensor_copy(dst, src)
                nc.vector.memset(v_aug[:, jt, :, :, HS:HS + 1], 1.0)

            # ---- yT accumulator (written during attention) ----
            # one yT tile per head pair (= per proj k-tile): keeps the
            # projection matmuls' dependencies per-pair, so proj k-tiles
            # 0..4 stream while the last pair's normalize chain drains
            yT_s = [ytpool.tile([128, T], BF16, tag=f"yT{hp}",
                                name=f"yT_{hp}")
                    for hp in range(NPAIR)]

            # prefetch projection weights (consumed only at the tail)
            wpts = []
            for wi, (off, w) in enumerate(((0, 512), (512, 256))):
                wpt = wppool.tile([128, KT, 512], BF16, tag="wp",
                                  name=f"wpt_{wi}")
                nc.sync.dma_start(wpt[:, :, 0:w], wp_r[:, :, off:off + w])
                wpts.append(wpt)

            # ---- per head-pair: QK projection then attention ----
            # Pair hp+1's QK projection is emitted between chunk 0 and
            # chunk 1 of pair hp: its ScalarE PSUM->SBUF copies land in
            # the ACT queue ahead of pair hp's chunk-1 exps, so they are
            # long done when pair hp+1's S^T matmuls need them.

            # Both heads of the pair run as one row-tiled unit: their
            # K=64 S^T matmuls go to PE row-groups 0-1 / 2-3 via
            # tile_position and stream concurrently.
            def emit_attn_chunk(hp, qk_t, c, tail=False):
                    njt = 4 * (c + 1)
                    # one P^T tile per j-tile, holding BOTH heads
                    # ([128, hl, 512]): written by one exp, so the two
                    # heads' S^T matmuls share readiness and stay
                    # adjacent in the schedule (required for the
                    # row-tiled pair to merge into one PE stream).
                    pts = [ptpool.tile([128, 2, 512], BF16, tag="pt",
                                       name=f"pt_{hp}_{c}_{j}")
                           for j in range(njt)]

                    # One [128, 1024] y accumulator per chunk: head0 =
                    # [y(0:64); sums(64)] in bank 0, head1 likewise in
                    # bank 1 (cols 512:1024). One tile -> one staging
                    # copy / one recip / one broadcast per chunk.
                    ybig = psB.tile([128, 1024], F32, tag="y",
                                    name=f"yps_{hp}_{c}")

                    def emit_pv(g):
                        # PV for the two j-tiles of group g (both heads)
                        for u in range(2):
                            jt = 2 * g + u
                            lo = max(0, (jt - 4 * c) * 128)
                            for hl in range(2):
                                out = ybig[0:HS + 1,
                                           512 * hl + lo:512 * hl + 512]
                                nc.tensor.matmul(
                                    out,
                                    v_aug[:, jt, hp, hl, :],
                                    pts[jt][:, hl, lo:512],
                                    start=(jt == 0),
                                    stop=(jt == njt - 1),
                                    skip_group_check=(jt > 0 or hl == 1),
                                )

                    # Software-pipelined emission: PV of group g-1 is
                    # interleaved between S^T groups so the in-order PE
                    # queue always has work that does not wait on the
                    # ScalarE exp (which drains the S^T PSUM slots).
                    for g in range(njt // 2):
                        for u in range(2):
                            jt = 2 * g + u
                            lo = max(0, (jt - 4 * c) * 128)
                            # hl0 in cols [lo:512] (bank 0), hl1 in
                            # [512+lo:1024] (bank 1) of one PSUM tile
                            st = psA.tile([128, 1024], F32, tag="big",
                                          name=f"st_{hp}_{c}_{jt}")
                            for hl in range(2):
                                base = 64 * hl
                                nc.tensor.matmul(
                                    st[:, hl * 512 + lo:(hl + 1) * 512],
                                    qk_t[base:base + 64, 1,
                                         jt * 128:(jt + 1) * 128],
                                    qk_t[base:base + 64, 0,
                                         c * 512 + lo:(c + 1) * 512],
                                    start=True, stop=True,
                                    tile_position=(base, 0),
                                )
                            # exp only the causally-needed columns [lo:512]
                            # of each head: cols [0:lo] are block-skipped
                            # (never written by S^T, never read by PV).
                            src = st[:].rearrange("p (a n) -> p a n", n=512)
                            nc.scalar.activation(
                                pts[jt][:, :, lo:512],
                                src[:, :, lo:512],
                                mybir.ActivationFunctionType.Exp,
                                scale=0.125,
                            )
                            if jt >= 4 * c:
                                # zero the j > i triangle of the diagonal
                                # P^T block, both heads in one DVE multiply
                                dlo = (jt - 4 * c) * 128
                                blk = pts[jt][:, :, dlo:dlo + 128]
                                nc.vector.tensor_mul(blk, blk, tri2_s[:])
                        if g >= 1:
                            emit_pv(g - 1)
                    emit_pv(njt // 2 - 1)
                    # normalize: yT[h rows, c cols] = y / sums. The recips
                    # read the two sums rows straight from PSUM (rows 64 /
                    # 63, partition-shifted to 0 — PSUM-source shifts work;
                    # SBUF-source ones don't), then ONE gpsimd broadcast
                    # fans 1/sums to all 128 partitions; the multiplies
                    # read y at matching partitions (0:64 / 64:128).
                    srow2 = smpool.tile([1, 2, 512], F32, tag="srow",
                                        name=f"srow_{hp}_{c}")
                    sbc2 = smpool.tile([128, 2, 512], F32, tag="sbc",
                                       name=f"sbc_{hp}_{c}")

                    def emit_srow_chain():
                        nc.vector.tensor_copy(
                            srow2[:],
                            ybig[HS:HS + 1, :].rearrange(
                                "p (a n) -> p a n", n=512))
                        nc.vector.reciprocal_approx_fast(srow2[:], srow2[:])
                        nc.gpsimd.partition_broadcast(sbc2[:], srow2[:])

                    if tail:
                        # end of the kernel: the output projection waits
                        # on this chain; multiply straight from PSUM (no
                        # later chunk needs the slot).
                        emit_srow_chain()
                        for hl in range(2):
                            nc.vector.tensor_mul(
                                yT_s[hp][64 * hl:64 * hl + 64,
                                         c * 512:(c + 1) * 512],
                                ybig[0:HS, 512 * hl:512 * hl + 512],
                                sbc2[0:HS, hl, :])
                    else:
                        # mid-kernel: the next chunk's PV waits on the y
                        # PSUM slot, so stage all of y out in ONE copy.
                        yst = smpool.tile([128, 1024], BF16, tag="yst",
                                          name=f"yst_{hp}_{c}")
                        nc.vector.tensor_copy(yst[:], ybig[:])
                        emit_srow_chain()
                        for hl in range(2):
                            nc.vector.tensor_mul(
                                yT_s[hp][64 * hl:64 * hl + 64,
                                         c * 512:(c + 1) * 512],
                                yst[0:HS, 512 * hl:512 * hl + 512],
                                sbc2[0:HS, hl, :])

            def emit_proj_unit(it, tail=False):
                # out tokens [it*128, (it+1)*128): both column groups.
                # Tail units drain their PSUM via ScalarE (idle after the
                # last exp) — on DVE the copy queues behind the last
                # pair's mask/normalize work and starves the psC ring.
                for (off, w), wpt in zip(((0, 512), (512, 256)), wpts):
                    ps = psC.tile([128, 512], F32, tag="pj")
                    for k in range(KT):
                        nc.tensor.matmul(
                            ps[:, 0:w],
                            yT_s[k][:, it * 128:(it + 1) * 128],
                            wpt[:, k, 0:w],
                            start=(k == 0), stop=(k == KT - 1),
                        )
                    ot = opool.tile([128, 512], F32, tag="ot")
                    if bias_proj:
                        nc.vector.tensor_add(ot[:, 0:w], ps[:, 0:w],
                                             bp_bc[:, off:off + w])
                    elif tail:
                        nc.scalar.copy(ot[:, 0:w], ps[:, 0:w])
                    else:
                        nc.vector.tensor_copy(ot[:, 0:w], ps[:, 0:w])
                    nc.sync.dma_start(out.ap()[it * 128:(it + 1) * 128,
                                               off:off + w], ot[:, 0:w])

            # Chunk-major: pass A covers queries 0-511 for all pairs
            # (with the QK projections interleaved); pass B covers
            # queries 512-1023, with the output projection for tokens
            # 0-511 interleaved between pairs as PE filler. Only the
            # projection of tokens 512-1023 remains after the last pair.
            qks = [qk_next]
            for hp in range(NPAIR):
                emit_attn_chunk(hp, qks[hp], 0)
                if hp + 1 < NPAIR:
                    qks.append(emit_qkmm(hp + 1))
            # proj unit for tokens of it=hp-1 is emitted one pair late:
            # its yT dependency region was written a whole pair ago, so
            # its matmuls are never gated on a fresh normalize chain.
            for hp in range(NPAIR):
                emit_attn_chunk(hp, qks[hp], 1, tail=(hp == NPAIR - 1))
                if hp < 4:
                    emit_proj_unit(hp)
            # Tail projection (tokens 512-1023) in two accumulation
            # phases: k=0..4 stream while the last pair's normalize
            # chain drains (their deps are pairs 0-4 only); the k=5
            # matmuls accumulate on top (start=False) once pair 5's yT
            # lands. The 8 phase-1 tiles use every PSUM bank — the
            # attention pools are all idle by now.
            tail_ps = []
            for u, it in enumerate(range(4, 8)):
                if u < 2 or u == 3:
                    pool, tag = (psA, "big") if u < 2 else (psB, "y")
                    big = pool.tile([128, 1024], F32, tag=tag,
                                    name=f"prj_{it}")
                    pss = [big[:, 0:512], big[:, 512:1024]]
                else:
                    pss = [psC.tile([128, 512], F32, tag="pj",
                                    name=f"prj_{it}_{g}")[:]
                           for g in range(2)]
                for pg, ((off, w), wpt) in zip(pss,
                                               zip(((0, 512), (512, 256)),
                                                   wpts)):
                    for k in range(KT - 1):
                        nc.tensor.matmul(
                            pg[:, 0:w],
                            yT_s[k][:, it * 128:(it + 1) * 128],
                            wpt[:, k, 0:w],
                            start=(k == 0), stop=(k == KT - 2),
                        )
                    tail_ps.append((it, off, w, wpt, pg))
            for it, off, w, wpt, pg in tail_ps:
                nc.tensor.matmul(
                    pg[:, 0:w],
                    yT_s[KT - 1][:, it * 128:(it + 1) * 128],
                    wpt[:, KT - 1, 0:w],
                    start=False, stop=True, skip_group_check=True,
                )
                ot = opool.tile([128, 512], F32, tag="ot")
                if bias_proj:
                    nc.vector.tensor_add(ot[:, 0:w], pg[:, 0:w],
                                         bp_bc[:, off:off + w])
                else:
                    nc.scalar.copy(ot[:, 0:w], pg[:, 0:w])
                nc.sync.dma_start(out.ap()[it * 128:(it + 1) * 128,
                                           off:off + w], ot[:, 0:w])

    nc.compile()
    return nc


def _get_program(bias_attn, bias_proj):
    key = (bias_attn, bias_proj)
    if key not in _cache:
        _cache[key] = _build_program(bias_attn, bias_proj)
    return _cache[key]


def _prep_inputs(x, w_attn, b_attn, w_proj, b_proj):
    x = np.asarray(x, dtype=np.float32)
    w_attn = np.asarray(w_attn, dtype=np.float32)
    b_attn = np.asarray(b_attn, dtype=np.float32)
    w_proj = np.asarray(w_proj, dtype=np.float32)
    b_proj = np.asarray(b_proj, dtype=np.float32)
    bias_attn = bool(np.any(b_attn))
    bias_proj = bool(np.any(b_proj))
    import ml_dtypes
    bf = ml_dtypes.bfloat16
    wqk = np.ascontiguousarray(w_attn[:, :2 * C]).astype(bf)
    # wv padded to 65-col head blocks [v(64) | 0] so the kernel's single
    # copy per (jt, group) lands v_aug in place (the zero col becomes the
    # ones column via an on-chip memset).
    wv_raw = w_attn[:, 2 * C:]
    wv = np.zeros((C, 780), dtype=np.float32)
    for h in range(NH):
        wv[:, h * 65:h * 65 + 64] = wv_raw[:, h * 64:h * 64 + 64]
    wv = np.ascontiguousarray(wv).astype(bf)
    wpb = w_proj.astype(bf)
    in_maps = []
    for b in range(NCORES):
        m = {
            "xT": np.ascontiguousarray(x[b].T).astype(bf),
            "wqk": wqk,
            "wv": wv,
            "wp": wpb,
        }
        if bias_attn:
            m["bqk"] = np.ascontiguousarray(b_attn[:2 * C])
            bv_raw = b_attn[2 * C:]
            bv = np.zeros((780,), dtype=np.float32)
            for h in range(NH):
                bv[h * 65:h * 65 + 64] = bv_raw[h * 64:h * 64 + 64]
            m["bv"] = bv
        if bias_proj:
            m["bp"] = b_proj
        in_maps.append(m)
    return in_maps, bias_attn, bias_proj


def run(x, w_attn, b_attn, w_proj, b_proj, trace=False, tmpdir=None):
    in_maps, bias_attn, bias_proj = _prep_inputs(
        x, w_attn, b_attn, w_proj, b_proj)
    nc = _get_program(bias_attn, bias_proj)
    res = run_bass_kernel_spmd(nc, in_maps, list(range(NCORES)),
                               trace=trace, tmpdir=tmpdir)
    out = np.stack([res.results[i]["out"] for i in range(NCORES)], axis=0)
    return out.astype(np.float32), res


def kernel(x, w_attn, b_attn, w_proj, b_proj):
    out, _ = run(x, w_attn, b_attn, w_proj, b_proj)
    return out

